# revision 8
# baseline (speedup 1.0000x reference)
"""Wall-clock benchmark of the bass kernel on 8 axon NeuronCores.

Mirrors bass2jax.run_bass_via_pjrt's multi-core path, but keeps the jitted
executable + device-resident inputs, and ping-pongs donated output buffers
so steady-state iterations move no host data.  Reports min/median over N
iterations, plus the same for a trivial NEFF to estimate dispatch overhead.
"""

import sys, time
import numpy as np
import ml_dtypes
import jax
from jax.sharding import Mesh, PartitionSpec, NamedSharding
try:
    from jax.experimental.shard_map import shard_map
except ImportError:
    from jax.shard_map import shard_map

import kernel as K
import concourse.bass as bass
import concourse.mybir as mybir
from concourse import bacc, bass2jax
from concourse.bass2jax import _bass_exec_p, install_neuronx_cc_hook, partition_id_tensor


def build_callable(nc, in_maps, n_cores):
    install_neuronx_cc_hook()
    assert nc.dbg_addr is None or not nc.dbg_callbacks
    if nc.dbg_addr is not None:
        in_maps = [
            {**m, nc.dbg_addr.name: np.zeros((1, 2), np.uint32)} for m in in_maps
        ]
    partition_name = nc.partition_id_tensor.name if nc.partition_id_tensor else None
    in_names, out_names, out_avals, zero_outs = [], [], [], []
    for alloc in nc.m.functions[0].allocations:
        if not isinstance(alloc, mybir.MemoryLocationSet):
            continue
        name = alloc.memorylocations[0].name
        if alloc.kind == "ExternalInput":
            if name != partition_name:
                in_names.append(name)
        elif alloc.kind == "ExternalOutput":
            out_names.append(name)
            shape = tuple(alloc.tensor_shape)
            dtype = mybir.dt.np(alloc.dtype)
            out_avals.append(jax.core.ShapedArray(shape, dtype))
            zero_outs.append(np.zeros(shape, dtype))
    n_params = len(in_names)
    n_outs = len(out_avals)
    in_names = in_names + out_names
    if partition_name is not None:
        in_names.append(partition_name)

    donate = tuple(range(n_params, n_params + n_outs))

    def _body(*args):
        operands = list(args)
        if partition_name is not None:
            operands.append(partition_id_tensor())
        outs = _bass_exec_p.bind(
            *operands,
            out_avals=tuple(out_avals),
            in_names=tuple(in_names),
            out_names=tuple(out_names),
            lowering_input_output_aliases=(),
            sim_require_finite=True,
            sim_require_nnan=True,
            nc=nc,
        )
        return tuple(outs)

    devices = jax.devices()[:n_cores]
    mesh = Mesh(np.asarray(devices), ("core",))
    in_specs = (PartitionSpec("core"),) * (n_params + n_outs)
    out_specs = (PartitionSpec("core"),) * len(out_names)
    sharded = jax.jit(
        shard_map(_body, mesh=mesh, in_specs=in_specs, out_specs=out_specs,
                  check_rep=False),
        donate_argnums=donate, keep_unused=True,
    )
    sh = NamedSharding(mesh, PartitionSpec("core"))
    per_core = [[np.asarray(m[name]) for name in in_names[:n_params]] for m in in_maps]
    concat_in = [
        jax.device_put(
            np.concatenate([per_core[c][i] for c in range(n_cores)], axis=0), sh)
        for i in range(n_params)
    ]
    concat_zeros = [
        jax.device_put(np.zeros((n_cores * z.shape[0], *z.shape[1:]), z.dtype), sh)
        for z in zero_outs
    ]
    return sharded, concat_in, concat_zeros, out_names, out_avals


def bench(nc, in_maps, n_cores, iters=30, label="kernel"):
    sharded, concat_in, bufs, out_names, out_avals = build_callable(
        nc, in_maps, n_cores)
    t0 = time.perf_counter()
    outs = jax.block_until_ready(sharded(*concat_in, *bufs))
    print(f"[{label}] first call (incl compile): {time.perf_counter()-t0:.1f}s",
          flush=True)
    times = []
    for _ in range(iters):
        t0 = time.perf_counter()
        outs = jax.block_until_ready(sharded(*concat_in, *outs))
        times.append((time.perf_counter() - t0) * 1e9)
    times = np.array(times)
    print(f"[{label}] min {times.min():.0f} ns  p50 {np.median(times):.0f} ns  "
          f"p10 {np.percentile(times,10):.0f} ns  max {times.max():.0f} ns",
          flush=True)
    return times, outs, out_names, out_avals


def build_trivial_nc():
    import concourse.tile as tile
    nc = bacc.Bacc()
    a = nc.declare_dram_parameter("a", [128, 128], mybir.dt.float32, isOutput=False)
    b = nc.declare_dram_parameter("b", [128, 128], mybir.dt.float32, isOutput=True)
    with tile.TileContext(nc) as tc:
        with tc.tile_pool(name="p", bufs=1) as p:
            t = p.tile([128, 128], mybir.dt.float32)
            nc.scalar.dma_start(out=t, in_=a[:])
            nc.scalar.dma_start(out=b[:], in_=t)
    nc.compile()
    return nc


if __name__ == "__main__":
    iters = int(sys.argv[1]) if len(sys.argv) > 1 else 30
    R = int(sys.argv[2]) if len(sys.argv) > 2 else 8

    d = np.load("/root/work/expected.npz") if __import__("os").path.exists(
        "/root/work/expected.npz") else np.load("/root/problem/expected.npz")
    x, U = d["x"], d["U"]
    xb = x.astype(ml_dtypes.bfloat16)
    up = np.zeros((K.LP, K.D), dtype=ml_dtypes.bfloat16)
    up[:K.L] = U.astype(ml_dtypes.bfloat16)
    in_maps = [{"x": np.ascontiguousarray(xb[i]), "u": up} for i in range(K.B)]

    nc = K.build_nc()
    t_k, outs, out_names, out_avals = bench(nc, in_maps, K.B, iters=iters,
                                            label="kernel(r=1)")
    ncR = K.build_nc(repeat=R)
    t_R, _, _, _ = bench(ncR, in_maps, K.B, iters=iters, label=f"kernel(r={R})")

    # correctness spot-check from the benchmarked outputs
    out_ref, alpha_ref = d["out"], d["alpha"]
    idx = {n: i for i, n in enumerate(out_names)}
    gout = np.asarray(outs[idx["out"]]).reshape(K.B, K.LP, K.D)[:, :K.L]
    galpha = np.asarray(outs[idx["alpha"]]).reshape(K.B, K.LP, K.S)[:, :K.L]
    rel_out = np.linalg.norm(gout - out_ref) / np.linalg.norm(out_ref)
    rel_alpha = np.linalg.norm(galpha - alpha_ref) / np.linalg.norm(alpha_ref)
    print(f"rel_out {rel_out:.5g}  rel_alpha {rel_alpha:.5g}")

    est = (t_R.min() - t_k.min()) / (R - 1)
    print(f"per-iteration exec estimate ((minR-min1)/{R-1}): {est:.0f} ns")
    print(f"HW exec time: {est:.0f} ns")


# revision 9
# speedup vs baseline: 6.2079x; 6.2079x over previous
"""Label-wise attention (CAML-style) on 8 TRN2 NeuronCores.

scores = U @ x^T        [B, L, S]
alpha  = softmax(scores, axis=S)
out    = alpha @ x      [B, L, D]
returns (out, alpha)

Sharding: batch B=8 across the 8 cores (1 batch each, full U per core).
No collectives needed.

Per-core kernel (bf16 matmuls, fp32 softmax/outputs):
  - x_b resident in SBUF in natural [s,d] layout (for alpha@x) and
    transposed [d,s] layout (for U@x^T), both bf16; transposed copies are
    made with batched DMA-xbar transposes (one instruction produces all
    128-column chunks: out[p, c, j] = in[j, c*128 + p]).
  - U streamed per 128-label tile, transposed on load the same way.
  - Per l-tile: PE matmul -> scores PSUM; ScalarE Exp (+accum_out row
    sums) -> bf16 exp tile; DVE scales to fp32 alpha -> HBM via SWDGE;
    SP issues one batched DMA-xbar transpose per scores quarter; PE
    matmul2 accumulates over 32 s-chunks; ScalarE copy-scale by
    1/rowsum -> fp32 out -> HBM.  Software-pipelined (matmul2 of tile
    i-1 is emitted after matmul1 of tile i) so PE never stalls on the
    exp/transpose chain.
"""

import sys
from contextlib import ExitStack

for _p in ("/opt/trn_rl_repo", "/opt/pypackages"):
    if _p not in sys.path:
        sys.path.insert(0, _p)

import numpy as np
import ml_dtypes

import concourse.bass as bass
import concourse.mybir as mybir
import concourse.tile as tile
from concourse import bacc

B, S, D, L = 8, 4096, 512, 8921
LP = 8960  # L padded to 70 * 128
BF16 = mybir.dt.bfloat16
F32 = mybir.dt.float32


def build_nc(s=S, n_lt=LP // 128, repeat=1):
    d = D
    n_dc = d // 128   # d-chunks of 128 (contraction for matmul1)
    n_sc = s // 128   # s-chunks of 128 (contraction for matmul2)
    qw = 1024 if s % 1024 == 0 else 512  # matmul1 PSUM chunk width
    n_q = s // qw
    lp = n_lt * 128

    nc = bacc.Bacc()
    x_d = nc.declare_dram_parameter("x", [s, d], BF16, isOutput=False)
    u_d = nc.declare_dram_parameter("u", [lp, d], BF16, isOutput=False)
    out_d = nc.declare_dram_parameter("out", [lp, d], F32, isOutput=True)
    alpha_d = nc.declare_dram_parameter("alpha", [lp, s], F32, isOutput=True)

    with tile.TileContext(nc) as tc, ExitStack() as ctx:
        const = ctx.enter_context(tc.tile_pool(name="const", bufs=1))
        utp = ctx.enter_context(tc.tile_pool(name="ut", bufs=3))
        expp = ctx.enter_context(tc.tile_pool(name="exp", bufs=2))
        expTp = ctx.enter_context(tc.tile_pool(name="expT", bufs=2))
        alphap = ctx.enter_context(tc.tile_pool(name="alpha", bufs=2))
        outp = ctx.enter_context(tc.tile_pool(name="out", bufs=3))
        statp = ctx.enter_context(tc.tile_pool(name="stat", bufs=3))
        ps1 = ctx.enter_context(tc.tile_pool(name="ps1", bufs=3, space="PSUM"))
        ps2 = ctx.enter_context(tc.tile_pool(name="ps2", bufs=2, space="PSUM"))

        # x natural: x_nat[p, c, :] = x[c*128 + p, :]
        x_nat = const.tile([128, n_sc, d], BF16)
        nc.gpsimd.dma_start(
            out=x_nat, in_=x_d[:].rearrange("(c p) d -> p c d", p=128)
        )
        # x transposed (one batched xpose): xT[p, dc, j] = x[j, dc*128 + p]
        xT = const.tile([128, n_dc, s], BF16)
        nc.sync.dma_start(out=xT, in_=x_d[:, :], transpose=True)

        pools = (utp, expp, expTp, alphap, outp, statp, ps1, ps2)
        import contextlib
        loop_cm = tc.For_i(0, repeat, 1) if repeat > 1 else contextlib.nullcontext()
        with loop_cm:
            _body(nc, n_lt, n_dc, n_sc, n_q, qw, s, d,
                  u_d, out_d, alpha_d, x_nat, xT, pools)
    nc.compile()
    return nc


def _body(nc, n_lt, n_dc, n_sc, n_q, qw, s, d,
          u_d, out_d, alpha_d, x_nat, xT, pools):
    (utp, expp, expTp, alphap, outp, statp, ps1, ps2) = pools
    prev = None
    for i in range(n_lt + 1):
        if i < n_lt:
            # UT[p, dc, j] = u[i*128 + j, dc*128 + p]  (one batched xpose)
            ut = utp.tile([128, n_dc, 128], BF16)
            nc.sync.dma_start(
                out=ut, in_=u_d[i * 128 : (i + 1) * 128, :], transpose=True
            )
            exp_t = expp.tile([128, s], BF16)
            expT_t = expTp.tile([128, n_sc, 128], BF16)
            psums = statp.tile([128, n_q], F32, tag="psums")
            nq_c = qw // 128  # xpose chunks per quarter
            for q in range(n_q):
                ps = ps1.tile([128, qw], F32)
                for h in range(qw // 512):
                    for dc in range(n_dc):
                        nc.tensor.matmul(
                            ps[:, h * 512 : (h + 1) * 512],
                            lhsT=ut[:, dc, :],
                            rhs=xT[:, dc, q * qw + h * 512 : q * qw + (h + 1) * 512],
                            start=(dc == 0),
                            stop=(dc == n_dc - 1),
                        )
                nc.scalar.activation(
                    out=exp_t[:, q * qw : (q + 1) * qw],
                    in_=ps[:],
                    func=mybir.ActivationFunctionType.Exp,
                    accum_out=psums[:, q : q + 1],
                )
                # expT[p, qc, j] = exp_t[j, qc*128 + p] for this quarter
                nc.sync.dma_start(
                    out=expT_t[:, q * nq_c : (q + 1) * nq_c, :],
                    in_=exp_t[:, q * qw : (q + 1) * qw],
                    transpose=True,
                )
            ssum = statp.tile([128, 1], F32, tag="ssum")
            recip = statp.tile([128, 1], F32, tag="recip")
            nc.vector.reduce_sum(out=ssum, in_=psums, axis=mybir.AxisListType.X)
            nc.vector.reciprocal(recip, ssum)
            alpha_t = alphap.tile([128, s], F32)
            nc.vector.tensor_scalar_mul(alpha_t, exp_t, recip)
            nc.gpsimd.dma_start(
                out=alpha_d[i * 128 : (i + 1) * 128, :], in_=alpha_t
            )
            cur = (expT_t, recip)
        if i > 0:
            expT_p, recip_p = prev
            po = ps2.tile([128, d], F32)
            for sc in range(n_sc):
                nc.tensor.matmul(
                    po,
                    lhsT=expT_p[:, sc, :],
                    rhs=x_nat[:, sc, :],
                    start=(sc == 0),
                    stop=(sc == n_sc - 1),
                )
            out_t = outp.tile([128, d], F32)
            nc.scalar.mul(out_t, po, recip_p)
            nc.gpsimd.dma_start(
                out=out_d[(i - 1) * 128 : i * 128, :], in_=out_t
            )
        if i < n_lt:
            prev = cur


def kernel(x: np.ndarray, U: np.ndarray):
    from concourse.bass_utils import run_bass_kernel_spmd

    xb = np.asarray(x).astype(ml_dtypes.bfloat16)
    up = np.zeros((LP, D), dtype=ml_dtypes.bfloat16)
    up[:L] = np.asarray(U).astype(ml_dtypes.bfloat16)

    nc = build_nc()
    in_maps = [{"x": np.ascontiguousarray(xb[i]), "u": up} for i in range(B)]
    res = run_bass_kernel_spmd(nc, in_maps, list(range(B))).results

    out = np.empty((B, L, D), dtype=np.float32)
    alpha = np.empty((B, L, S), dtype=np.float32)
    for i in range(B):
        out[i] = res[i]["out"][:L]
        alpha[i] = res[i]["alpha"][:L]
    return out, alpha
